# revision 35
# baseline (speedup 1.0000x reference)
"""Trainium2 Bass kernel for fused attention block (QKV proj + RoPE + SDPA + out proj).

Reference computation (B=4, S=2048, HID=2048, H=16, D=128, fp32):
    qkv = hidden @ w_qkv; q,k,v split per head
    q,k = RoPE(q,k, cos,sin)
    attn = softmax(q k^T / sqrt(D)) v          (per batch, head)
    out  = attn.reshape(B,S,H*D) @ w_o

Sharding (8 cores): core c -> (batch b=c//2, head-group g=c%2 of 8 heads).
Each core computes a partial output [S, HID] over its 8 heads; the host sums
the two head-group partials per batch.

Per-core kernel. All matmul operands are fp16 (1 cyc/row on PE at any free
size, same rate as fp32r, but half the DMA/SBUF cost); PSUM accumulation and
the final output are fp32. Error budget: every fp16 rounding site is ~5e-4
relative; end-to-end measured ~1e-3, gate is 2e-2.

  Phase 1: QKV projection from X^T (host-pretransposed fp16), RoPE fused at
           PSUM eviction (rotate_half = partition-swap via two SBUF-SBUF
           copies; sign and 1/sqrt(D) folded into host-prepared cos/sin).
           X^T streamed in quarter tiles (bufs=4), each quarter split into 4
           DMAs so it spreads across queues. Both halves interleave Q,K per
           head so weight loads hide under the other pass; half 2 runs V
           first, then per-head (Q,K) so attention head h starts as soon as
           ITS q/k are written (fp16, via DRAM scratch).
  Phase 2: per head: scores^T [sk,128 x sq,512] = K_chunk @ Q^T_block; Exp on
           ACT pair-batched over two PSUM banks ([128,1024] -> ACT 8.6us per
           512-q block, under PE's ~9.0us so PE never idles and never drops
           p-state); P~^T fp16. Denominator: 10-op fp16 DVE add tree (2x
           mode) to 6 partials + 6 accumulating ones-matmuls; those 6 PE
           matmuls for block i-1 are interleaved into block i's score/PV
           stream. 1/d via reciprocal_approx_fast (DVE custom op, ~0.6us).
           O^T += V^T @ P~^T; normalize at eviction; O^T fp16 -> DRAM.
  Phase 3: out = O_flat @ w_o via lhsT = O^T slices (fp16), rhs = w_o fp16
           (prefetched into SBUF during phase 2), fp32 eviction, host sums
           the two head-group partials.

Softmax is computed without max subtraction: scores are O(1)-scaled
(|s| < ~15 even at 5+ sigma), so exp() is well within fp16 range (e^11 max).
"""

import os
import sys
import types

sys.path.insert(0, "/opt/trn_rl_repo")

import numpy as np

B, S, HID = 4, 2048, 2048
H, D = 16, 128
HG = 8            # heads per core (head-group)
NCORES = 8
SB = 512          # s-block (matmul free dim)
NSB = S // SB     # 4
NKT = HID // 128  # 16 k-tiles over hidden
NSK = S // 128    # 16 key chunks

_STATE = {}
LAST_RESULTS = None


def _ensure_ntff_hook():
    """bass_utils wants antenv.axon_hooks for NTFF tracing under axon; this
    container's antenv lacks it. Register the ctypes-backed hook."""
    try:
        from antenv import axon_hooks  # noqa: F401
        return
    except ImportError:
        pass
    import antenv
    from trn_agent_boot.trn_boot import _ntff_profile_via_ctypes

    mod = types.ModuleType("antenv.axon_hooks")
    _hook = [None]
    mod.set_axon_ntff_profile_hook = lambda h: _hook.__setitem__(0, h)
    mod.get_axon_ntff_profile_hook = lambda: _hook[0]
    sys.modules["antenv.axon_hooks"] = mod
    antenv.axon_hooks = mod
    mod.set_axon_ntff_profile_hook(
        _ntff_profile_via_ctypes("/opt/axon/libaxon_pjrt.so")
    )


def _build():
    import concourse.mybir as mybir
    import concourse.tile as tile
    from concourse import bacc

    F32 = mybir.dt.float32
    F32R = mybir.dt.float32r
    F16 = mybir.dt.float16
    EXP = mybir.ActivationFunctionType.Exp

    nc = bacc.Bacc(None, target_bir_lowering=False, debug=False)

    x_t = nc.dram_tensor("x_t", [HID, S], F16, kind="ExternalInput")
    w_q = nc.dram_tensor("w_q", [128, NKT, HG * 128], F16, kind="ExternalInput")
    w_k = nc.dram_tensor("w_k", [128, NKT, HG * 128], F16, kind="ExternalInput")
    w_v = nc.dram_tensor("w_v", [128, NKT, HG * 128], F16, kind="ExternalInput")
    cos_q = nc.dram_tensor("cos_q", [128, S], F16, kind="ExternalInput")
    sin_q = nc.dram_tensor("sin_q", [128, S], F16, kind="ExternalInput")
    cos_k = nc.dram_tensor("cos_k", [128, S], F16, kind="ExternalInput")
    sin_k = nc.dram_tensor("sin_k", [128, S], F16, kind="ExternalInput")
    ones_in = nc.dram_tensor("ones_in", [128, 128], F16, kind="ExternalInput")
    w_o = nc.dram_tensor("w_o", [128, HG, HID], F16, kind="ExternalInput")
    out_p = nc.dram_tensor("out_p", [S, HID], F32, kind="ExternalOutput")

    SH = S // 2   # half
    SQ = S // 4   # quarter (x^T streaming granularity)

    with tile.TileContext(nc) as tc:
        with tc.tile_pool(name="dram", bufs=1, space="DRAM") as dr:
            # Per-head / per-pair scratch tensors give Tile fine-grained
            # cross-phase deps, so attention head h starts as soon as ITS
            # q/k are written.
            q_ropes = [dr.tile([128, S], F16, name=f"q_rope{c}") for c in range(HG)]
            k_ropes = [dr.tile([128, S], F16, name=f"k_rope{c}") for c in range(HG)]
            v_pairs = [dr.tile([S, 512], F16, name=f"v_quad{j}") for j in range(2)]
            o_t = dr.tile([HG, 128, S], F16)

            with (
                tc.tile_pool(name="p2h", bufs=2) as hp,
                # pt/sum pools live outside phase 1's scope: their SBUF is
                # disjoint from p1's pools, so phase 2's first exps don't
                # wait on the p1 tail draining.
                tc.tile_pool(name="p2pt", bufs=10) as ptp,
                tc.tile_pool(name="p2sm", bufs=12) as smp,
            ):
                # ---------------- Phase 1: QKV projection + RoPE ----------
                with (
                    tc.tile_pool(name="p1xt", bufs=3) as xtp,
                    tc.tile_pool(name="p1w", bufs=2) as wp,
                    tc.tile_pool(name="p1vw", bufs=3) as vwp,
                    tc.tile_pool(name="p1cs", bufs=1) as csp,
                    tc.tile_pool(name="p1evq", bufs=4) as evq,
                    tc.tile_pool(name="p1evs", bufs=3) as evs,
                    tc.tile_pool(name="p1qf", bufs=3) as qfp,
                    tc.tile_pool(name="p1vev", bufs=2) as vevp,
                    tc.tile_pool(name="p1ps", bufs=5, space="PSUM") as ps1,
                    tc.tile_pool(name="p1vps", bufs=2, space="PSUM") as vps,
                ):
                    def load_quarter(qi):
                        xt = xtp.tile([128, NKT, SQ], F16, tag="xt")
                        for kg in range(4):  # 4 sub-DMAs -> parallel queues
                            nc.sync.dma_start(
                                xt[:, 4 * kg : 4 * (kg + 1), :],
                                x_t[4 * kg * 128 : 4 * (kg + 1) * 128,
                                    qi * SQ : (qi + 1) * SQ].rearrange(
                                    "(ko p) s -> p ko s", p=128),
                            )
                        return xt

                    def load_wv(vc):
                        wvc = vwp.tile([128, NKT, 512], F16, tag="wv")
                        nc.sync.dma_start(
                            wvc[:], w_v[:, :, vc * 512 : (vc + 1) * 512])
                        return wvc

                    def v_pass(sh, xts, wv0=None, wv1=None, after_first=None):
                        for vc in range(2):
                            pre = (wv0, wv1)[vc]
                            wvc = pre if pre is not None else load_wv(vc)
                            for ss in range(SH // 128):
                                if after_first is not None and (vc, ss) == (0, 4):
                                    # big prefetches issued after the first V
                                    # chain so they can't stall it in the DMA
                                    # pipeline
                                    after_first()
                                    after_first = None
                                xt = xts[ss // 4]
                                lo = (ss % 4) * 128
                                ps = vps.tile([128, 512], F32, tag="ps_v")
                                for kt in range(NKT):
                                    nc.tensor.matmul(
                                        ps[:],
                                        xt[:, kt, lo : lo + 128],
                                        wvc[:, kt, :],
                                        start=(kt == 0),
                                        stop=(kt == NKT - 1),
                                    )
                                vt = vevp.tile([128, 512], F16, tag="vt")
                                nc.vector.tensor_copy(vt[:], ps[:])
                                row = sh * SH + ss * 128
                                nc.gpsimd.dma_start(
                                    v_pairs[vc][row : row + 128, :], vt[:]
                                )

                    def qk_head(sh, c, w_dram, cs_tiles, ropes, xts, wtag,
                                preloaded=None):
                        # one head's projection + RoPE over one half
                        cqf, sqf = cs_tiles
                        if preloaded is not None:
                            wc = preloaded
                        else:
                            wc = wp.tile([128, NKT, 128], F16, tag=wtag)
                            nc.sync.dma_start(
                                wc[:], w_dram[:, :, c * 128 : (c + 1) * 128]
                            )
                        for sb in range(2):  # two SB=512 blocks per half
                            xt = xts[sb]
                            gsl = slice(sh * SH + sb * SB, sh * SH + (sb + 1) * SB)
                            lsl = slice(sb * SB, (sb + 1) * SB)
                            ps = ps1.tile([128, SB], F32, tag="ps_qk")
                            for kt in range(NKT):
                                nc.tensor.matmul(
                                    ps[:],
                                    wc[:, kt, :],
                                    xt[:, kt, :],
                                    start=(kt == 0),
                                    stop=(kt == NKT - 1),
                                )
                            qt = evq.tile([128, SB], F32R, tag="qt")
                            nc.scalar.copy(qt[:], ps[:])
                            qs = evs.tile([128, SB], F32R, tag="qs")
                            nc.gpsimd.dma_start(qs[0:64, :], qt[64:128, :])
                            nc.gpsimd.dma_start(qs[64:128, :], qt[0:64, :])
                            nc.vector.tensor_mul(qt[:], qt[:], cqf[:, lsl])
                            nc.vector.tensor_mul(qs[:], qs[:], sqf[:, lsl])
                            qf = qfp.tile([128, SB], F16, tag="qf")
                            nc.vector.tensor_add(qf[:], qt[:], qs[:])
                            nc.gpsimd.dma_start(ropes[c][:, gsl], qf[:])

                    def load_cs(sh):
                        hsl = slice(sh * SH, (sh + 1) * SH)
                        out = []
                        for nm, t in (("cq", cos_q), ("sq", sin_q),
                                      ("ck", cos_k), ("sk", sin_k)):
                            f = csp.tile([128, SH], F16, tag=nm)
                            nc.sync.dma_start(f[:], t[:, hsl])
                            out.append(f)
                        return (out[0], out[1]), (out[2], out[3])

                    # ---- half 1: per-head (Q,K) pairs, then V ----
                    # head-0 weights first so the first matmul isn't queued
                    # behind the x/cos DMA traffic.
                    wq0 = wp.tile([128, NKT, 128], F16, tag="wq")
                    nc.sync.dma_start(wq0[:], w_q[:, :, 0:128])
                    wk0 = wp.tile([128, NKT, 128], F16, tag="wk")
                    nc.sync.dma_start(wk0[:], w_k[:, :, 0:128])
                    xt0 = load_quarter(0)
                    xt1 = load_quarter(1)
                    cs_q1, cs_k1 = load_cs(0)
                    # half-1 V weights issued early so they are resident long
                    # before the V pass starts.
                    wv0 = load_wv(0)
                    wv1 = load_wv(1)
                    for c in range(HG):
                        qk_head(0, c, w_q, cs_q1, q_ropes, (xt0, xt1), "wq",
                                preloaded=wq0 if c == 0 else None)
                        qk_head(0, c, w_k, cs_k1, k_ropes, (xt0, xt1), "wk",
                                preloaded=wk0 if c == 0 else None)
                    xt23 = []

                    def prefetch_half2():
                        xt23.append(load_quarter(2))
                        xt23.append(load_quarter(3))

                    v_pass(0, (xt0, xt1), wv0=wv0, wv1=wv1,
                           after_first=prefetch_half2)

                    # ---- half 2: V first, then per-head (Q,K) pairs ----
                    cs_q2, cs_k2 = load_cs(1)
                    xt2, xt3 = xt23
                    v_pass(1, (xt2, xt3))
                    for c in range(HG):
                        qk_head(1, c, w_q, cs_q2, q_ropes, (xt2, xt3), "wq")
                        qk_head(1, c, w_k, cs_k2, k_ropes, (xt2, xt3), "wk")

                # ---------------- Phase 2: attention -----------------
                with (
                    tc.tile_pool(name="p3wo", bufs=1) as wop,
                    tc.tile_pool(name="p2r", bufs=2) as rp,
                    tc.tile_pool(name="p2ev", bufs=3) as evp2,
                    tc.tile_pool(name="p2c", bufs=1) as cp,
                ):
                  with (
                    tc.tile_pool(name="p2ps_s", bufs=2, space="PSUM") as ps_s,
                    tc.tile_pool(name="p2ps_d", bufs=1, space="PSUM") as ps_d,
                    tc.tile_pool(name="p2ps_o", bufs=2, space="PSUM") as ps_o,
                  ):
                    ones = cp.tile([128, 128], F16, tag="ones")
                    nc.sync.dma_start(ones[:], ones_in[:])
                    wo = wop.tile([128, HG, HID], F16, tag="wo")

                    # pending: (sums[6], pso, h, qsl, psd) of the previous
                    # (h, sqb) block
                    pending = [None]

                    def emit_ones(j):
                        # j-th accumulating denominator matmul of the pending
                        # block (j = 0..5)
                        if pending[0] is None:
                            return
                        sums, pso, h, qsl, psd = pending[0]
                        nc.tensor.matmul(
                            psd[:], ones[:], sums[j][:],
                            start=(j == 0), stop=(j == 5),
                        )
                        if j == 5:
                            rec = rp.tile([128, SB], F32, tag="rec")
                            nc.vector.reciprocal_approx_fast(rec[:], psd[:])
                            ote = evp2.tile([128, SB], F16, tag="ote")
                            nc.vector.tensor_mul(ote[:], pso[:], rec[:])
                            nc.gpsimd.dma_start(o_t[h, :, qsl], ote[:])
                            pending[0] = None

                    for h in range(HG):
                        qT = hp.tile([128, S], F16, tag="qT")
                        nc.sync.dma_start(qT[:], q_ropes[h])
                        kT = hp.tile([128, S], F16, tag="kT")
                        nc.sync.dma_start(kT[:], k_ropes[h])
                        vh = hp.tile([128, NSK, 128], F16, tag="vh")
                        nc.sync.dma_start(
                            vh[:],
                            v_pairs[h // 4][:, (h % 4) * 128 : (h % 4 + 1) * 128]
                            .rearrange("(so p) d -> p so d", p=128),
                        )
                        if h == 5:
                            # 4MB output-projection weight prefetch, late and
                            # in slices so it never crowds the DMA pipeline
                            for hh in range(HG):
                                nc.sync.dma_start(wo[:, hh, :], w_o[:, hh, :])
                        for sqb in range(NSB):
                            qsl = slice(sqb * SB, (sqb + 1) * SB)
                            pso = ps_o.tile([128, SB], F32, tag="ps_o")
                            pts = []   # 8 pair tiles [128, 2, SB] fp16
                            # PE stream per j: scores pair j, one denominator
                            # matmul of block i-1, PV pair j-2.
                            for j in range(8):
                                pss = ps_s.tile([128, 2 * SB], F32, tag="ps_s")
                                for u in range(2):
                                    nc.tensor.matmul(
                                        pss[:, u * SB : (u + 1) * SB],
                                        kT[:, (2 * j + u) * 128
                                           : (2 * j + u + 1) * 128],
                                        qT[:, qsl],
                                        start=True,
                                        stop=True,
                                    )
                                pt = ptp.tile([128, 2, SB], F16, tag="pt")
                                nc.scalar.activation(
                                    pt[:].rearrange("p u s -> p (u s)"),
                                    pss[:], EXP,
                                )
                                pts.append(pt)
                                if j >= 2:
                                    emit_ones(j - 2)
                                    for u in range(2):
                                        sk = 2 * (j - 2) + u
                                        nc.tensor.matmul(
                                            pso[:], vh[:, sk, :],
                                            pts[j - 2][:, u, :],
                                            start=(sk == 0), stop=False,
                                        )
                            for j in (6, 7):
                                for u in range(2):
                                    sk = 2 * j + u
                                    nc.tensor.matmul(
                                        pso[:], vh[:, sk, :], pts[j][:, u, :],
                                        start=False, stop=(sk == NSK - 1),
                                    )
                            # fp16 add tree (DVE 2x mode) down to 6 tiles:
                            # pairs 0..5 collapse in-tile; pairs 6,7 add into
                            # tiles 4,5.
                            sums = []
                            for j in range(6):
                                t = smp.tile([128, SB], F16, tag="tsum")
                                nc.vector.tensor_add(
                                    t[:], pts[j][:, 0, :], pts[j][:, 1, :]
                                )
                                sums.append(t)
                            for j in (6, 7):
                                t = sums[j - 2]
                                nc.vector.tensor_add(t[:], t[:], pts[j][:, 0, :])
                                nc.vector.tensor_add(t[:], t[:], pts[j][:, 1, :])
                            psd = ps_d.tile([128, SB], F32, tag="ps_d")
                            pending[0] = (sums, pso, h, qsl, psd)
                    for j in range(6):
                        emit_ones(j)

                  # ------------ Phase 3: output projection ------------
                  # (same SBUF scope as phase 2: wo was prefetched above)
                  with (
                    tc.tile_pool(name="p3i", bufs=4) as otcp,
                    tc.tile_pool(name="p3o", bufs=4) as outp,
                    tc.tile_pool(name="p3ps", bufs=4, space="PSUM") as ps3,
                  ):
                    for sc in range(S // 128):
                        otc = otcp.tile([128, HG, 128], F16, tag="otc")
                        nc.sync.dma_start(
                            otc[:],
                            o_t[:, :, sc * 128 : (sc + 1) * 128].rearrange(
                                "h p s -> p h s"
                            ),
                        )
                        for nb in range(HID // SB):
                            ps = ps3.tile([128, SB], F32, tag="ps3")
                            for h in range(HG):
                                nc.tensor.matmul(
                                    ps[:],
                                    otc[:, h, :],
                                    wo[:, h, nb * SB : (nb + 1) * SB],
                                    start=(h == 0),
                                    stop=(h == HG - 1),
                                )
                            ot = outp.tile([128, SB], F32, tag="out")
                            nc.vector.tensor_copy(ot[:], ps[:])
                            nc.sync.dma_start(
                                out_p[sc * 128 : (sc + 1) * 128,
                                      nb * SB : (nb + 1) * SB],
                                ot[:],
                            )

    nc.compile()
    return nc


def _get_nc():
    if "nc" not in _STATE:
        _STATE["nc"] = _build()
    return _STATE["nc"]


def kernel(hidden_states, cos, sin, w_qkv, w_o):
    global LAST_RESULTS
    from concourse.bass_utils import run_bass_kernel_spmd

    trace = os.environ.get("KERNEL_TRACE", "") == "1"
    if trace:
        _ensure_ntff_hook()

    hidden_states = np.asarray(hidden_states, dtype=np.float32)
    cos = np.asarray(cos, dtype=np.float32)
    sin = np.asarray(sin, dtype=np.float32)
    w_qkv = np.asarray(w_qkv, dtype=np.float32)
    w_o = np.asarray(w_o, dtype=np.float32)

    cos_t = np.ascontiguousarray(cos.T)                      # [128, S]
    sin_t = np.ascontiguousarray(sin.T)
    sin_rot = np.concatenate([-sin_t[:64], sin_t[64:]], axis=0)
    scale = np.float32(1.0 / np.sqrt(D))
    cos_qh = np.ascontiguousarray(cos_t * scale).astype(np.float16)
    sin_qh = np.ascontiguousarray(sin_rot * scale).astype(np.float16)
    cos_kh = cos_t.astype(np.float16)
    sin_kh = sin_rot.astype(np.float16)
    ones = np.ones((128, 128), np.float16)

    def ktile(w):  # [HID, N] -> [128, NKT, N] fp16
        n = w.shape[1]
        return np.ascontiguousarray(
            w.reshape(NKT, 128, n).transpose(1, 0, 2)).astype(np.float16)

    in_maps = []
    for c in range(NCORES):
        b, g = divmod(c, 2)
        cs = slice(g * HG * D, (g + 1) * HG * D)
        wq = ktile(w_qkv[:, 0:H * D][:, cs])
        wk = ktile(w_qkv[:, H * D:2 * H * D][:, cs])
        wv = ktile(w_qkv[:, 2 * H * D:3 * H * D][:, cs])
        wo_c = w_o[cs, :]
        wo_r = np.ascontiguousarray(
            wo_c.reshape(HG, 128, HID).transpose(1, 0, 2)).astype(np.float16)
        in_maps.append({
            "x_t": np.ascontiguousarray(hidden_states[b].T).astype(np.float16),
            "w_q": wq, "w_k": wk, "w_v": wv,
            "cos_q": cos_qh, "sin_q": sin_qh,
            "cos_k": cos_kh, "sin_k": sin_kh,
            "ones_in": ones,
            "w_o": wo_r,
        })

    nc = _get_nc()
    res = run_bass_kernel_spmd(
        nc, in_maps, core_ids=list(range(NCORES)), trace=trace
    )
    LAST_RESULTS = res

    out = np.empty((B, S, HID), np.float32)
    for b in range(B):
        out[b] = res.results[2 * b]["out_p"] + res.results[2 * b + 1]["out_p"]
    return out


# revision 36
# speedup vs baseline: 1.0987x; 1.0987x over previous
"""Trainium2 Bass kernel for fused attention block (QKV proj + RoPE + SDPA + out proj).

Reference computation (B=4, S=2048, HID=2048, H=16, D=128, fp32):
    qkv = hidden @ w_qkv; q,k,v split per head
    q,k = RoPE(q,k, cos,sin)
    attn = softmax(q k^T / sqrt(D)) v          (per batch, head)
    out  = attn.reshape(B,S,H*D) @ w_o

Sharding (8 cores): core c -> (batch b=c//2, head-group g=c%2 of 8 heads).
Each core computes a partial output [S, HID] over its 8 heads; the host sums
the two head-group partials per batch.

Per-core kernel. All matmul operands are fp16 (1 cyc/row on PE at any free
size, same rate as fp32r, but half the DMA/SBUF cost); PSUM accumulation and
the final output are fp32. Error budget: every fp16 rounding site is ~5e-4
relative; end-to-end measured ~1e-3, gate is 2e-2.

  Phase 1: QKV projection from X^T (host-pretransposed fp16), RoPE fused at
           PSUM eviction (rotate_half = partition-swap via two SBUF-SBUF
           copies; sign and 1/sqrt(D) folded into host-prepared cos/sin).
           X^T streamed in quarter tiles (bufs=4), each quarter split into 4
           DMAs so it spreads across queues. Both halves interleave Q,K per
           head so weight loads hide under the other pass; half 2 runs V
           first, then per-head (Q,K) so attention head h starts as soon as
           ITS q/k are written (fp16, via DRAM scratch).
  Phase 2: per head: scores^T [sk,128 x sq,512] = K_chunk @ Q^T_block; Exp on
           ACT pair-batched over two PSUM banks ([128,1024] -> ACT 8.6us per
           512-q block, under PE's ~9.0us so PE never idles and never drops
           p-state); P~^T fp16. Denominator: 10-op fp16 DVE add tree (2x
           mode) to 6 partials + 6 accumulating ones-matmuls; those 6 PE
           matmuls for block i-1 are interleaved into block i's score/PV
           stream. 1/d via reciprocal_approx_fast (DVE custom op, ~0.6us).
           O^T += V^T @ P~^T; normalize at eviction; O^T fp16 -> DRAM.
  Phase 3: out = O_flat @ w_o via lhsT = O^T slices (fp16), rhs = w_o fp16
           (prefetched into SBUF during phase 2), fp32 eviction, host sums
           the two head-group partials.

Softmax is computed without max subtraction: scores are O(1)-scaled
(|s| < ~15 even at 5+ sigma), so exp() is well within fp16 range (e^11 max).
"""

import os
import sys
import types

sys.path.insert(0, "/opt/trn_rl_repo")

import numpy as np

B, S, HID = 4, 2048, 2048
H, D = 16, 128
HG = 8            # heads per core (head-group)
NCORES = 8
SB = 512          # s-block (matmul free dim)
NSB = S // SB     # 4
NKT = HID // 128  # 16 k-tiles over hidden
NSK = S // 128    # 16 key chunks

_STATE = {}
LAST_RESULTS = None


def _ensure_ntff_hook():
    """bass_utils wants antenv.axon_hooks for NTFF tracing under axon; this
    container's antenv lacks it. Register the ctypes-backed hook."""
    try:
        from antenv import axon_hooks  # noqa: F401
        return
    except ImportError:
        pass
    import antenv
    from trn_agent_boot.trn_boot import _ntff_profile_via_ctypes

    mod = types.ModuleType("antenv.axon_hooks")
    _hook = [None]
    mod.set_axon_ntff_profile_hook = lambda h: _hook.__setitem__(0, h)
    mod.get_axon_ntff_profile_hook = lambda: _hook[0]
    sys.modules["antenv.axon_hooks"] = mod
    antenv.axon_hooks = mod
    mod.set_axon_ntff_profile_hook(
        _ntff_profile_via_ctypes("/opt/axon/libaxon_pjrt.so")
    )


def _build():
    import concourse.mybir as mybir
    import concourse.tile as tile
    from concourse import bacc

    F32 = mybir.dt.float32
    F32R = mybir.dt.float32r
    F16 = mybir.dt.float16
    EXP = mybir.ActivationFunctionType.Exp

    nc = bacc.Bacc(None, target_bir_lowering=False, debug=False)

    x_t = nc.dram_tensor("x_t", [HID, S], F16, kind="ExternalInput")
    w_q = nc.dram_tensor("w_q", [128, NKT, HG * 128], F16, kind="ExternalInput")
    w_k = nc.dram_tensor("w_k", [128, NKT, HG * 128], F16, kind="ExternalInput")
    w_v = nc.dram_tensor("w_v", [128, NKT, HG * 128], F16, kind="ExternalInput")
    cos_q = nc.dram_tensor("cos_q", [128, S], F16, kind="ExternalInput")
    sin_q = nc.dram_tensor("sin_q", [128, S], F16, kind="ExternalInput")
    cos_k = nc.dram_tensor("cos_k", [128, S], F16, kind="ExternalInput")
    sin_k = nc.dram_tensor("sin_k", [128, S], F16, kind="ExternalInput")
    ones_in = nc.dram_tensor("ones_in", [128, 128], F16, kind="ExternalInput")
    w_o = nc.dram_tensor("w_o", [128, HG, HID], F16, kind="ExternalInput")
    out_p = nc.dram_tensor("out_p", [S, HID], F32, kind="ExternalOutput")

    SH = S // 2   # half
    SQ = S // 4   # quarter (x^T streaming granularity)

    with tile.TileContext(nc) as tc:
        with tc.tile_pool(name="dram", bufs=1, space="DRAM") as dr:
            # Per-head / per-pair scratch tensors give Tile fine-grained
            # cross-phase deps, so attention head h starts as soon as ITS
            # q/k are written.
            q_ropes = [dr.tile([128, S], F16, name=f"q_rope{c}") for c in range(HG)]
            k_ropes = [dr.tile([128, S], F16, name=f"k_rope{c}") for c in range(HG)]
            v_pairs = [dr.tile([S, 512], F16, name=f"v_quad{j}") for j in range(2)]
            o_t = dr.tile([HG, 128, S], F16)

            with (
                tc.tile_pool(name="p2h", bufs=2) as hp,
                # pt/sum pools live outside phase 1's scope: their SBUF is
                # disjoint from p1's pools, so phase 2's first exps don't
                # wait on the p1 tail draining.
                tc.tile_pool(name="p2pt", bufs=10) as ptp,
                tc.tile_pool(name="p2sm", bufs=12) as smp,
            ):
                # ---------------- Phase 1: QKV projection + RoPE ----------
                with (
                    tc.tile_pool(name="p1xt", bufs=3) as xtp,
                    tc.tile_pool(name="p1w", bufs=2) as wp,
                    tc.tile_pool(name="p1vw", bufs=3) as vwp,
                    tc.tile_pool(name="p1cs", bufs=1) as csp,
                    tc.tile_pool(name="p1evq", bufs=4) as evq,
                    tc.tile_pool(name="p1evs", bufs=4) as evs,
                    tc.tile_pool(name="p1qf", bufs=4) as qfp,
                    tc.tile_pool(name="p1vev", bufs=3) as vevp,
                    tc.tile_pool(name="p1ps", bufs=5, space="PSUM") as ps1,
                    tc.tile_pool(name="p1vps", bufs=2, space="PSUM") as vps,
                ):
                    def load_quarter(qi):
                        xt = xtp.tile([128, NKT, SQ], F16, tag="xt")
                        for kg in range(4):  # 4 sub-DMAs -> parallel queues
                            nc.sync.dma_start(
                                xt[:, 4 * kg : 4 * (kg + 1), :],
                                x_t[4 * kg * 128 : 4 * (kg + 1) * 128,
                                    qi * SQ : (qi + 1) * SQ].rearrange(
                                    "(ko p) s -> p ko s", p=128),
                            )
                        return xt

                    def load_wv(vc):
                        wvc = vwp.tile([128, NKT, 512], F16, tag="wv")
                        nc.sync.dma_start(
                            wvc[:], w_v[:, :, vc * 512 : (vc + 1) * 512])
                        return wvc

                    def v_pass(sh, xts, wv0=None, wv1=None, after_first=None):
                        for vc in range(2):
                            pre = (wv0, wv1)[vc]
                            wvc = pre if pre is not None else load_wv(vc)
                            for ss in range(SH // 128):
                                if after_first is not None and (vc, ss) == (0, 4):
                                    # big prefetches issued after the first V
                                    # chain so they can't stall it in the DMA
                                    # pipeline
                                    after_first()
                                    after_first = None
                                xt = xts[ss // 4]
                                lo = (ss % 4) * 128
                                ps = vps.tile([128, 512], F32, tag="ps_v")
                                for kt in range(NKT):
                                    nc.tensor.matmul(
                                        ps[:],
                                        xt[:, kt, lo : lo + 128],
                                        wvc[:, kt, :],
                                        start=(kt == 0),
                                        stop=(kt == NKT - 1),
                                    )
                                vt = vevp.tile([128, 512], F16, tag="vt")
                                nc.vector.tensor_copy(vt[:], ps[:])
                                row = sh * SH + ss * 128
                                nc.gpsimd.dma_start(
                                    v_pairs[vc][row : row + 128, :], vt[:]
                                )

                    def qk_head(sh, c, w_dram, cs_tiles, ropes, xts, wtag,
                                preloaded=None):
                        # one head's projection + RoPE over one half
                        cqf, sqf = cs_tiles
                        if preloaded is not None:
                            wc = preloaded
                        else:
                            wc = wp.tile([128, NKT, 128], F16, tag=wtag)
                            nc.sync.dma_start(
                                wc[:], w_dram[:, :, c * 128 : (c + 1) * 128]
                            )
                        for sb in range(2):  # two SB=512 blocks per half
                            xt = xts[sb]
                            gsl = slice(sh * SH + sb * SB, sh * SH + (sb + 1) * SB)
                            lsl = slice(sb * SB, (sb + 1) * SB)
                            ps = ps1.tile([128, SB], F32, tag="ps_qk")
                            for kt in range(NKT):
                                nc.tensor.matmul(
                                    ps[:],
                                    wc[:, kt, :],
                                    xt[:, kt, :],
                                    start=(kt == 0),
                                    stop=(kt == NKT - 1),
                                )
                            qt = evq.tile([128, SB], F32R, tag="qt")
                            nc.scalar.copy(qt[:], ps[:])
                            qs = evs.tile([128, SB], F32R, tag="qs")
                            nc.gpsimd.dma_start(qs[0:64, :], qt[64:128, :])
                            nc.gpsimd.dma_start(qs[64:128, :], qt[0:64, :])
                            nc.vector.tensor_mul(qt[:], qt[:], cqf[:, lsl])
                            nc.vector.tensor_mul(qs[:], qs[:], sqf[:, lsl])
                            qf = qfp.tile([128, SB], F16, tag="qf")
                            nc.vector.tensor_add(qf[:], qt[:], qs[:])
                            nc.gpsimd.dma_start(ropes[c][:, gsl], qf[:])

                    def load_cs(sh):
                        hsl = slice(sh * SH, (sh + 1) * SH)
                        out = []
                        for nm, t in (("cq", cos_q), ("sq", sin_q),
                                      ("ck", cos_k), ("sk", sin_k)):
                            f = csp.tile([128, SH], F16, tag=nm)
                            nc.sync.dma_start(f[:], t[:, hsl])
                            out.append(f)
                        return (out[0], out[1]), (out[2], out[3])

                    # ---- half 1: per-head (Q,K) pairs, then V ----
                    # head-0 weights first so the first matmul isn't queued
                    # behind the x/cos DMA traffic.
                    wq0 = wp.tile([128, NKT, 128], F16, tag="wq")
                    nc.sync.dma_start(wq0[:], w_q[:, :, 0:128])
                    wk0 = wp.tile([128, NKT, 128], F16, tag="wk")
                    nc.sync.dma_start(wk0[:], w_k[:, :, 0:128])
                    xt0 = load_quarter(0)
                    xt1 = load_quarter(1)
                    cs_q1, cs_k1 = load_cs(0)
                    # half-1 V weights issued early so they are resident long
                    # before the V pass starts.
                    wv0 = load_wv(0)
                    wv1 = load_wv(1)
                    for c in range(HG):
                        qk_head(0, c, w_q, cs_q1, q_ropes, (xt0, xt1), "wq",
                                preloaded=wq0 if c == 0 else None)
                        qk_head(0, c, w_k, cs_k1, k_ropes, (xt0, xt1), "wk",
                                preloaded=wk0 if c == 0 else None)
                    xt23 = []

                    def prefetch_half2():
                        xt23.append(load_quarter(2))
                        xt23.append(load_quarter(3))

                    v_pass(0, (xt0, xt1), wv0=wv0, wv1=wv1,
                           after_first=prefetch_half2)

                    # ---- half 2: V first, then per-head (Q,K) pairs ----
                    cs_q2, cs_k2 = load_cs(1)
                    xt2, xt3 = xt23
                    v_pass(1, (xt2, xt3))
                    for c in range(HG):
                        qk_head(1, c, w_q, cs_q2, q_ropes, (xt2, xt3), "wq")
                        qk_head(1, c, w_k, cs_k2, k_ropes, (xt2, xt3), "wk")

                # ---------------- Phase 2: attention -----------------
                with (
                    tc.tile_pool(name="p3wo", bufs=1) as wop,
                    tc.tile_pool(name="p2r", bufs=2) as rp,
                    tc.tile_pool(name="p2ev", bufs=3) as evp2,
                    tc.tile_pool(name="p2c", bufs=1) as cp,
                ):
                  with (
                    tc.tile_pool(name="p2ps_s", bufs=2, space="PSUM") as ps_s,
                    tc.tile_pool(name="p2ps_d", bufs=1, space="PSUM") as ps_d,
                    tc.tile_pool(name="p2ps_o", bufs=2, space="PSUM") as ps_o,
                  ):
                    ones = cp.tile([128, 128], F16, tag="ones")
                    nc.sync.dma_start(ones[:], ones_in[:])
                    wo = wop.tile([128, HG, HID], F16, tag="wo")

                    # pending: (sums[6], pso, h, qsl, psd) of the previous
                    # (h, sqb) block
                    pending = [None]

                    def emit_ones(j):
                        # j-th accumulating denominator matmul of the pending
                        # block (j = 0..5)
                        if pending[0] is None:
                            return
                        sums, pso, h, qsl, psd = pending[0]
                        nc.tensor.matmul(
                            psd[:], ones[:], sums[j][:],
                            start=(j == 0), stop=(j == 5),
                        )
                        if j == 5:
                            rec = rp.tile([128, SB], F32, tag="rec")
                            nc.vector.reciprocal_approx_fast(rec[:], psd[:])
                            ote = evp2.tile([128, SB], F16, tag="ote")
                            nc.vector.tensor_mul(ote[:], pso[:], rec[:])
                            nc.gpsimd.dma_start(o_t[h, :, qsl], ote[:])
                            pending[0] = None

                    for h in range(HG):
                        qT = hp.tile([128, S], F16, tag="qT")
                        nc.sync.dma_start(qT[:], q_ropes[h])
                        kT = hp.tile([128, S], F16, tag="kT")
                        nc.sync.dma_start(kT[:], k_ropes[h])
                        vh = hp.tile([128, NSK, 128], F16, tag="vh")
                        nc.sync.dma_start(
                            vh[:],
                            v_pairs[h // 4][:, (h % 4) * 128 : (h % 4 + 1) * 128]
                            .rearrange("(so p) d -> p so d", p=128),
                        )
                        if h == 5:
                            # 4MB output-projection weight prefetch, late and
                            # in slices so it never crowds the DMA pipeline
                            for hh in range(HG):
                                nc.sync.dma_start(wo[:, hh, :], w_o[:, hh, :])
                        for sqb in range(NSB):
                            qsl = slice(sqb * SB, (sqb + 1) * SB)
                            pso = ps_o.tile([128, SB], F32, tag="ps_o")
                            pts = []   # 8 pair tiles [128, 2, SB] fp16
                            # PE stream per j: scores pair j, one denominator
                            # matmul of block i-1, PV pair j-2.
                            for j in range(8):
                                pss = ps_s.tile([128, 2 * SB], F32, tag="ps_s")
                                for u in range(2):
                                    nc.tensor.matmul(
                                        pss[:, u * SB : (u + 1) * SB],
                                        kT[:, (2 * j + u) * 128
                                           : (2 * j + u + 1) * 128],
                                        qT[:, qsl],
                                        start=True,
                                        stop=True,
                                    )
                                pt = ptp.tile([128, 2, SB], F16, tag="pt")
                                nc.scalar.activation(
                                    pt[:].rearrange("p u s -> p (u s)"),
                                    pss[:], EXP,
                                )
                                pts.append(pt)
                                if j >= 2:
                                    emit_ones(j - 2)
                                    for u in range(2):
                                        sk = 2 * (j - 2) + u
                                        nc.tensor.matmul(
                                            pso[:], vh[:, sk, :],
                                            pts[j - 2][:, u, :],
                                            start=(sk == 0), stop=False,
                                        )
                            for j in (6, 7):
                                for u in range(2):
                                    sk = 2 * j + u
                                    nc.tensor.matmul(
                                        pso[:], vh[:, sk, :], pts[j][:, u, :],
                                        start=False, stop=(sk == NSK - 1),
                                    )
                            # fp16 add tree (DVE 2x mode) down to 6 tiles:
                            # pairs 0..5 collapse in-tile; pairs 6,7 add into
                            # tiles 4,5.
                            sums = []
                            for j in range(6):
                                t = smp.tile([128, SB], F16, tag="tsum")
                                nc.vector.tensor_add(
                                    t[:], pts[j][:, 0, :], pts[j][:, 1, :]
                                )
                                sums.append(t)
                            for j in (6, 7):
                                t = sums[j - 2]
                                nc.vector.tensor_add(t[:], t[:], pts[j][:, 0, :])
                                nc.vector.tensor_add(t[:], t[:], pts[j][:, 1, :])
                            psd = ps_d.tile([128, SB], F32, tag="ps_d")
                            pending[0] = (sums, pso, h, qsl, psd)
                    for j in range(6):
                        emit_ones(j)

                  # ------------ Phase 3: output projection ------------
                  # (same SBUF scope as phase 2: wo was prefetched above)
                  with (
                    tc.tile_pool(name="p3i", bufs=4) as otcp,
                    tc.tile_pool(name="p3o", bufs=4) as outp,
                    tc.tile_pool(name="p3ps", bufs=4, space="PSUM") as ps3,
                  ):
                    for sc in range(S // 128):
                        otc = otcp.tile([128, HG, 128], F16, tag="otc")
                        nc.sync.dma_start(
                            otc[:],
                            o_t[:, :, sc * 128 : (sc + 1) * 128].rearrange(
                                "h p s -> p h s"
                            ),
                        )
                        for nb in range(HID // SB):
                            ps = ps3.tile([128, SB], F32, tag="ps3")
                            for h in range(HG):
                                nc.tensor.matmul(
                                    ps[:],
                                    otc[:, h, :],
                                    wo[:, h, nb * SB : (nb + 1) * SB],
                                    start=(h == 0),
                                    stop=(h == HG - 1),
                                )
                            ot = outp.tile([128, SB], F32, tag="out")
                            nc.vector.tensor_copy(ot[:], ps[:])
                            nc.sync.dma_start(
                                out_p[sc * 128 : (sc + 1) * 128,
                                      nb * SB : (nb + 1) * SB],
                                ot[:],
                            )

    nc.compile()
    return nc


def _get_nc():
    if "nc" not in _STATE:
        _STATE["nc"] = _build()
    return _STATE["nc"]


def kernel(hidden_states, cos, sin, w_qkv, w_o):
    global LAST_RESULTS
    from concourse.bass_utils import run_bass_kernel_spmd

    trace = os.environ.get("KERNEL_TRACE", "") == "1"
    if trace:
        _ensure_ntff_hook()

    hidden_states = np.asarray(hidden_states, dtype=np.float32)
    cos = np.asarray(cos, dtype=np.float32)
    sin = np.asarray(sin, dtype=np.float32)
    w_qkv = np.asarray(w_qkv, dtype=np.float32)
    w_o = np.asarray(w_o, dtype=np.float32)

    cos_t = np.ascontiguousarray(cos.T)                      # [128, S]
    sin_t = np.ascontiguousarray(sin.T)
    sin_rot = np.concatenate([-sin_t[:64], sin_t[64:]], axis=0)
    scale = np.float32(1.0 / np.sqrt(D))
    cos_qh = np.ascontiguousarray(cos_t * scale).astype(np.float16)
    sin_qh = np.ascontiguousarray(sin_rot * scale).astype(np.float16)
    cos_kh = cos_t.astype(np.float16)
    sin_kh = sin_rot.astype(np.float16)
    ones = np.ones((128, 128), np.float16)

    def ktile(w):  # [HID, N] -> [128, NKT, N] fp16
        n = w.shape[1]
        return np.ascontiguousarray(
            w.reshape(NKT, 128, n).transpose(1, 0, 2)).astype(np.float16)

    in_maps = []
    for c in range(NCORES):
        b, g = divmod(c, 2)
        cs = slice(g * HG * D, (g + 1) * HG * D)
        wq = ktile(w_qkv[:, 0:H * D][:, cs])
        wk = ktile(w_qkv[:, H * D:2 * H * D][:, cs])
        wv = ktile(w_qkv[:, 2 * H * D:3 * H * D][:, cs])
        wo_c = w_o[cs, :]
        wo_r = np.ascontiguousarray(
            wo_c.reshape(HG, 128, HID).transpose(1, 0, 2)).astype(np.float16)
        in_maps.append({
            "x_t": np.ascontiguousarray(hidden_states[b].T).astype(np.float16),
            "w_q": wq, "w_k": wk, "w_v": wv,
            "cos_q": cos_qh, "sin_q": sin_qh,
            "cos_k": cos_kh, "sin_k": sin_kh,
            "ones_in": ones,
            "w_o": wo_r,
        })

    nc = _get_nc()
    res = run_bass_kernel_spmd(
        nc, in_maps, core_ids=list(range(NCORES)), trace=trace
    )
    LAST_RESULTS = res

    out = np.empty((B, S, HID), np.float32)
    for b in range(B):
        out[b] = res.results[2 * b]["out_p"] + res.results[2 * b + 1]["out_p"]
    return out
